# revision 6
# baseline (speedup 1.0000x reference)
"""Depthwise 3D transposed conv (stride 2, k=4, SAME) on 8 trn2 NeuronCores.

x: (4, 32, 32, 32, 256) f32, filters: (4, 4, 4, 1, 256) f32
y: (4, 64, 64, 64, 256) f32

Sharding: 8 cores = (batch n in 4) x (d-halves in 2). Zero communication.

Math: polyphase decomposition of the stride-2 transposed conv. Per dim,
output parity p uses taps (delta, k):
  p=0: y[2m]   = f[3] x[m-1] + f[1] x[m]
  p=1: y[2m+1] = f[2] x[m]   + f[0] x[m+1]
So each output element is a sum of exactly 8 taps (2 per dim).

On-chip layout: channels (128 per group) on partitions, spatial on free dim.
Each (out-plane, polyphase, cgroup, a-half) tile of [128c, 16a, 32b] outputs
is computed either:
 - PE path: 8 accumulating float32r matmuls with per-tap diagonal 128x128
   weights (depthwise == diag matmul), evacuated PSUM->SBUF by ScalarE, or
 - DVE path: ScalarE per-partition-scale multiply for tap 0, then 7 fused
   scalar_tensor_tensor MACs on VectorE.
Output accumulates in polyphase-major SBUF slabs; the store DMA interleaves
polyphases back to NDHWC via its access pattern.
"""
import sys

sys.path.insert(0, "/opt/trn_rl_repo")

from contextlib import ExitStack

import numpy as np

import concourse.bass as bass  # noqa: F401  (registers engine classes)
import concourse.tile as tile
from concourse import bacc, mybir
from concourse.bass_utils import run_bass_kernel_spmd

F32 = mybir.dt.float32
F32R = mybir.dt.float32r
AOP = mybir.AluOpType

N_CORES = 8
# per-dim taps: parity -> [(delta, k), ...]
TAPS = {0: [(-1, 3), (0, 1)], 1: [(0, 2), (1, 0)]}
# fraction of tiles routed to the DVE assist path: every 4th
DVE_MOD, DVE_RES = 4, 3

_PROG = None


def _build_program():
    nc = bacc.Bacc(
        "TRN2", target_bir_lowering=False, debug=False, num_devices=N_CORES
    )
    # x: channels-first (d, c, h, w); y: polyphase-major (do, c, ph, pw, a, b).
    # Host code does these transposes; they keep every DMA a <=3-dim
    # contiguous-final-dim access pattern (the DMA AP requirement).
    x_d = nc.declare_dram_parameter("x", [18, 256, 32, 32], F32, isOutput=False)
    wd_d = nc.declare_dram_parameter("wdiag", [128, 2, 64, 128], F32, isOutput=False)
    fl_d = nc.declare_dram_parameter("fils", [128, 128], F32, isOutput=False)
    y_d = nc.declare_dram_parameter("y", [32, 256, 2, 2, 32, 32], F32, isOutput=True)

    with ExitStack() as ctx:
        tc = ctx.enter_context(tile.TileContext(nc))
        wpool = ctx.enter_context(tc.tile_pool(name="wpool", bufs=1))
        xpool = ctx.enter_context(tc.tile_pool(name="xpool", bufs=8))
        opool = ctx.enter_context(tc.tile_pool(name="opool", bufs=4))
        ppool = ctx.enter_context(tc.tile_pool(name="ppool", bufs=6, space="PSUM"))

        wd = wpool.tile([128, 2, 64, 128], F32R)
        nc.sync.dma_start(out=wd[:], in_=wd_d[:, :, :, :].bitcast(F32R))
        fl = wpool.tile([128, 128], F32)
        nc.sync.dma_start(out=fl[:], in_=fl_d[:, :])

        xcache = {}

        def get_plane(pl, cg):
            key = (pl, cg)
            if key not in xcache:
                t = xpool.tile([128, 34, 34], F32R, tag="xp")
                # zero whole tile (halo border); interior DMA-overwritten
                nc.gpsimd.memset(t[:].bitcast(F32), 0.0)
                nc.sync.dma_start(
                    out=t[:, 1:33, 1:33],
                    in_=x_d[pl, cg * 128 : (cg + 1) * 128, :, :].bitcast(F32R),
                )
                xcache[key] = t
            return xcache[key]

        tile_i = 0
        for l in range(32):
            m_loc = l // 2 + 1  # local index of plane m (tile plane 0 == m-1)
            pd = l % 2
            for cg in range(2):
                cs = cg * 128
                # polyphase-major accumulator: [c, ph, pw, a, b]
                ot = opool.tile([128, 2, 2, 32, 32], F32, tag="out")
                for ph, pw in ((0, 0), (0, 1), (1, 0), (1, 1)):
                    for ah in range(2):
                        a0 = ah * 16
                        taps = [
                            (dd, kd, dh, kh, dw, kw)
                            for (dd, kd) in TAPS[pd]
                            for (dh, kh) in TAPS[ph]
                            for (dw, kw) in TAPS[pw]
                        ]
                        out_view = ot[:, ph, pw, a0 : a0 + 16, :]
                        use_dve = tile_i % DVE_MOD == DVE_RES
                        if use_dve:
                            for t_i, (dd, kd, dh, kh, dw, kw) in enumerate(taps):
                                xt = get_plane(m_loc + dd, cg)
                                win = xt[
                                    :,
                                    1 + a0 + dh : 1 + a0 + dh + 16,
                                    1 + dw : 1 + dw + 32,
                                ].bitcast(F32)
                                tap = kd * 16 + kh * 4 + kw
                                wsc = fl[:, cg * 64 + tap : cg * 64 + tap + 1]
                                if t_i == 0:
                                    # out = win * w  (ScalarE, per-partition scale)
                                    nc.scalar.mul(out_view, win, wsc)
                                else:
                                    # out = win * w + out  (VectorE fused MAC)
                                    nc.vector.scalar_tensor_tensor(
                                        out_view,
                                        win,
                                        wsc,
                                        out_view,
                                        AOP.mult,
                                        AOP.add,
                                    )
                        else:
                            ps = ppool.tile([128, 16, 32], F32, tag="ps")
                            for t_i, (dd, kd, dh, kh, dw, kw) in enumerate(taps):
                                xt = get_plane(m_loc + dd, cg)
                                win = xt[
                                    :,
                                    1 + a0 + dh : 1 + a0 + dh + 16,
                                    1 + dw : 1 + dw + 32,
                                ]
                                tap = kd * 16 + kh * 4 + kw
                                wap = wd[:, cg, tap, :]
                                nc.tensor.matmul(
                                    ps[:],
                                    wap,
                                    win,
                                    start=(t_i == 0),
                                    stop=(t_i == len(taps) - 1),
                                )
                            nc.scalar.copy(out_view, ps[:])
                        tile_i += 1
                # store: contiguous [128, 4096] block; host un-interleaves
                nc.sync.dma_start(
                    out=y_d[l, cs : cs + 128, :, :, :, :], in_=ot[:]
                )
    nc.compile()
    return nc


def _get_program():
    global _PROG
    if _PROG is None:
        _PROG = _build_program()
    return _PROG


def _make_in_maps(x, filters):
    x = np.ascontiguousarray(np.asarray(x), dtype=np.float32)
    filters = np.asarray(filters, dtype=np.float32)
    ftap = filters[:, :, :, 0, :].reshape(64, 256)  # [kd*16+kh*4+kw, c]
    fils = np.zeros((128, 128), np.float32)  # [c_in_group, cg*64 + tap]
    for cg in range(2):
        fils[:, cg * 64 : (cg + 1) * 64] = ftap[:, cg * 128 : (cg + 1) * 128].T
    wdiag = np.zeros((128, 2, 64, 128), np.float32)  # [c, cg, tap, c_out]
    idx = np.arange(128)
    for cg in range(2):
        wdiag[idx, cg, :, idx] = ftap[:, cg * 128 : (cg + 1) * 128].T

    in_maps = []
    for core in range(N_CORES):
        n, h = core // 2, core % 2
        lo = 16 * h - 1
        planes = np.zeros((18, 32, 32, 256), np.float32)
        s0, s1 = max(lo, 0), min(16 * h + 17, 32)
        planes[s0 - lo : s1 - lo] = x[n, s0:s1]
        planes = np.ascontiguousarray(planes.transpose(0, 3, 1, 2))
        in_maps.append({"x": planes, "wdiag": wdiag, "fils": fils})
    return in_maps


def kernel(x, filters):
    nc = _get_program()
    in_maps = _make_in_maps(x, filters)
    res = run_bass_kernel_spmd(nc, in_maps, list(range(N_CORES)))
    y = np.empty((4, 64, 64, 64, 256), np.float32)
    for core in range(N_CORES):
        n, h = core // 2, core % 2
        yc = res.results[core]["y"]  # (32, 256, 2, 2, 32, 32)
        yc = yc.transpose(0, 4, 2, 5, 3, 1).reshape(32, 64, 64, 256)
        y[n, 32 * h : 32 * h + 32] = yc
    return y


# revision 8
# speedup vs baseline: 1.0442x; 1.0442x over previous
"""Depthwise 3D transposed conv (stride 2, k=4, SAME) on 8 trn2 NeuronCores.

x: (4, 32, 32, 32, 256) f32, filters: (4, 4, 4, 1, 256) f32
y: (4, 64, 64, 64, 256) f32

Sharding: 8 cores = (batch n in 4) x (d-halves in 2). Zero communication.

Math: polyphase decomposition of the stride-2 transposed conv. Per dim,
output parity p uses taps (delta, k):
  p=0: y[2m]   = f[3] x[m-1] + f[1] x[m]
  p=1: y[2m+1] = f[2] x[m]   + f[0] x[m+1]
So each output element is a sum of exactly 8 taps (2 per dim).

On-chip layout: channels (128 per group) on partitions, spatial on free dim.
Each (out-plane, polyphase, cgroup, a-half) tile of [128c, 16a, 32b] outputs
is computed either:
 - PE path: 8 accumulating float32r matmuls with per-tap diagonal 128x128
   weights (depthwise == diag matmul), evacuated PSUM->SBUF by ScalarE, or
 - DVE path: ScalarE per-partition-scale multiply for tap 0, then 7 fused
   scalar_tensor_tensor MACs on VectorE.
Output accumulates in polyphase-major SBUF slabs; the store DMA interleaves
polyphases back to NDHWC via its access pattern.
"""
import sys

sys.path.insert(0, "/opt/trn_rl_repo")

from contextlib import ExitStack

import numpy as np

import concourse.bass as bass  # noqa: F401  (registers engine classes)
import concourse.tile as tile
from concourse import bacc, mybir
from concourse.bass_utils import run_bass_kernel_spmd

F32 = mybir.dt.float32
F32R = mybir.dt.float32r
F16 = mybir.dt.float16
AOP = mybir.AluOpType

N_CORES = 8
# per-dim taps: parity -> [(delta, k), ...]
TAPS = {0: [(-1, 3), (0, 1)], 1: [(0, 2), (1, 0)]}
# DVE assist share: units u with (u*DVE_NUM) % DVE_DEN < DVE_NUM go to DVE
DVE_NUM, DVE_DEN = 9, 32

_PROG = None


def _build_program():
    nc = bacc.Bacc(
        "TRN2", target_bir_lowering=False, debug=False, num_devices=N_CORES
    )
    # x: channels-first (d, c, h, w); y: polyphase-major (do, c, ph, pw, a, b).
    # Host code does these transposes; they keep every DMA a <=3-dim
    # contiguous-final-dim access pattern (the DMA AP requirement).
    x_d = nc.declare_dram_parameter("x", [18, 256, 32, 32], F32, isOutput=False)
    wd_d = nc.declare_dram_parameter("wdiag", [128, 2, 64, 128], F32, isOutput=False)
    fl_d = nc.declare_dram_parameter("fils", [128, 128], F32, isOutput=False)
    y_d = nc.declare_dram_parameter("y", [32, 256, 2, 2, 32, 32], F32, isOutput=True)

    with ExitStack() as ctx:
        tc = ctx.enter_context(tile.TileContext(nc))
        wpool = ctx.enter_context(tc.tile_pool(name="wpool", bufs=1))
        xpool = ctx.enter_context(tc.tile_pool(name="xpool", bufs=8))
        opool = ctx.enter_context(tc.tile_pool(name="opool", bufs=4))
        ppool = ctx.enter_context(tc.tile_pool(name="ppool", bufs=6, space="PSUM"))

        wd = wpool.tile([128, 2, 64, 128], F32R)
        nc.sync.dma_start(out=wd[:], in_=wd_d[:, :, :, :].bitcast(F32R))
        fl = wpool.tile([128, 128], F32)
        nc.sync.dma_start(out=fl[:], in_=fl_d[:, :])

        xcache = {}

        def get_plane(pl, cg):
            key = (pl, cg)
            if key not in xcache:
                t = xpool.tile([128, 34, 34], F32R, tag="xp")
                # zero whole tile (halo border); interior DMA-overwritten
                nc.gpsimd.memset(t[:].bitcast(F32), 0.0)
                nc.sync.dma_start(
                    out=t[:, 1:33, 1:33],
                    in_=x_d[pl, cg * 128 : (cg + 1) * 128, :, :].bitcast(F32R),
                )
                xcache[key] = t
            return xcache[key]

        tile_i = 0
        for l in range(32):
            m_loc = l // 2 + 1  # local index of plane m (tile plane 0 == m-1)
            pd = l % 2
            for cg in range(2):
                cs = cg * 128
                # polyphase-major accumulator: [c, ph, pw, a, b]
                ot = opool.tile([128, 2, 2, 32, 32], F32, tag="out")
                for ph, pw in ((0, 0), (0, 1), (1, 0), (1, 1)):
                    taps = [
                        (dd, kd, dh, kh, dw, kw)
                        for (dd, kd) in TAPS[pd]
                        for (dh, kh) in TAPS[ph]
                        for (dw, kw) in TAPS[pw]
                    ]
                    use_dve = (tile_i * DVE_NUM) % DVE_DEN < DVE_NUM
                    if use_dve:
                        # full [128, 32, 32] unit on ScalarE (tap 0) + VectorE
                        out_view = ot[:, ph, pw, :, :]
                        for t_i, (dd, kd, dh, kh, dw, kw) in enumerate(taps):
                            xt = get_plane(m_loc + dd, cg)
                            win = xt[
                                :, 1 + dh : 1 + dh + 32, 1 + dw : 1 + dw + 32
                            ].bitcast(F32)
                            tap = kd * 16 + kh * 4 + kw
                            wsc = fl[:, cg * 64 + tap : cg * 64 + tap + 1]
                            if t_i == 0:
                                nc.scalar.mul(out_view, win, wsc)
                            else:
                                nc.vector.scalar_tensor_tensor(
                                    out_view,
                                    win,
                                    wsc,
                                    out_view,
                                    AOP.mult,
                                    AOP.add,
                                )
                    else:
                        for ah in range(2):
                            a0 = ah * 16
                            out_view = ot[:, ph, pw, a0 : a0 + 16, :]
                            ps = ppool.tile([128, 16, 32], F32, tag="ps")
                            for t_i, (dd, kd, dh, kh, dw, kw) in enumerate(taps):
                                xt = get_plane(m_loc + dd, cg)
                                win = xt[
                                    :,
                                    1 + a0 + dh : 1 + a0 + dh + 16,
                                    1 + dw : 1 + dw + 32,
                                ]
                                tap = kd * 16 + kh * 4 + kw
                                wap = wd[:, cg, tap, :]
                                nc.tensor.matmul(
                                    ps[:],
                                    wap,
                                    win,
                                    start=(t_i == 0),
                                    stop=(t_i == len(taps) - 1),
                                )
                            nc.scalar.copy(out_view, ps[:])
                    tile_i += 1
                # store: contiguous [128, 4096] block; host un-interleaves
                nc.sync.dma_start(
                    out=y_d[l, cs : cs + 128, :, :, :, :], in_=ot[:]
                )
    nc.compile()
    return nc


def _get_program():
    global _PROG
    if _PROG is None:
        _PROG = _build_program()
    return _PROG


def _make_in_maps(x, filters):
    x = np.ascontiguousarray(np.asarray(x), dtype=np.float32)
    filters = np.asarray(filters, dtype=np.float32)
    ftap = filters[:, :, :, 0, :].reshape(64, 256)  # [kd*16+kh*4+kw, c]
    fils = np.zeros((128, 128), np.float32)  # [c_in_group, cg*64 + tap]
    for cg in range(2):
        fils[:, cg * 64 : (cg + 1) * 64] = ftap[:, cg * 128 : (cg + 1) * 128].T
    wdiag = np.zeros((128, 2, 64, 128), np.float32)  # [c, cg, tap, c_out]
    idx = np.arange(128)
    for cg in range(2):
        wdiag[idx, cg, :, idx] = ftap[:, cg * 128 : (cg + 1) * 128].T

    in_maps = []
    for core in range(N_CORES):
        n, h = core // 2, core % 2
        lo = 16 * h - 1
        planes = np.zeros((18, 32, 32, 256), np.float32)
        s0, s1 = max(lo, 0), min(16 * h + 17, 32)
        planes[s0 - lo : s1 - lo] = x[n, s0:s1]
        planes = np.ascontiguousarray(planes.transpose(0, 3, 1, 2))
        in_maps.append({"x": planes, "wdiag": wdiag, "fils": fils})
    return in_maps


def kernel(x, filters):
    nc = _get_program()
    in_maps = _make_in_maps(x, filters)
    res = run_bass_kernel_spmd(nc, in_maps, list(range(N_CORES)))
    y = np.empty((4, 64, 64, 64, 256), np.float32)
    for core in range(N_CORES):
        n, h = core // 2, core % 2
        yc = res.results[core]["y"]  # (32, 256, 2, 2, 32, 32)
        yc = yc.transpose(0, 4, 2, 5, 3, 1).reshape(32, 64, 64, 256)
        y[n, 32 * h : 32 * h + 32] = yc
    return y


# revision 10
# speedup vs baseline: 1.0480x; 1.0036x over previous
"""Depthwise 3D transposed conv (stride 2, k=4, SAME) on 8 trn2 NeuronCores.

x: (4, 32, 32, 32, 256) f32, filters: (4, 4, 4, 1, 256) f32
y: (4, 64, 64, 64, 256) f32

Sharding: 8 cores = (batch n in 4) x (d-halves in 2). Zero communication.

Math: polyphase decomposition of the stride-2 transposed conv. Per dim,
output parity p uses taps (delta, k):
  p=0: y[2m]   = f[3] x[m-1] + f[1] x[m]
  p=1: y[2m+1] = f[2] x[m]   + f[0] x[m+1]
Each output element is a sum of exactly 8 taps (2 per dim).

Compute (all TensorE, float32r = 1 cycle/row):
- Contraction folds the 2 d-taps: input tile partitions hold a PLANE PAIR,
  p = j*64 + cc <- (plane k+j, channel block cc of 64).
- Weight columns fold 2 OUTPUT PLANES: both output planes l=2k-1 (d-parity
  1) and l=2k (d-parity 0) read the same plane pair (k, k+1) with the same
  (dh, dw) window shifts, so a [128, 128] weight matrix with columns
  (r*64 + c'), W[(j,cc),(r,c')] = delta(cc,c') * F[kd(j, parity(r)), kh,
  kw, c'], computes partial sums for two planes in one matmul.
Each matmul thus covers 4 of the 8 taps for 2x64 channels x 512 positions
(256 useful MACs/cycle); 4 (dh,dw) taps accumulate per PSUM bank. ScalarE
evacuates PSUM->SBUF. ~2176 matmuls/core total.

Host pre-pairs planes (xp[k] = planes (k, k+1)) so every tile load is one
full-width 128-partition DMA with 4KB-contiguous per-partition runs. The
output accumulates in (plane-pair, polyphase)-major slabs stored as
contiguous [128, 16KB] DMAs; the host un-interleaves (and drops the two
out-of-range boundary plane slots).
"""
import sys

sys.path.insert(0, "/opt/trn_rl_repo")

from contextlib import ExitStack

import numpy as np

import concourse.bass as bass  # noqa: F401  (registers engine classes)
import concourse.tile as tile
from concourse import bacc, mybir
from concourse.bass_utils import run_bass_kernel_spmd

F32 = mybir.dt.float32
F32R = mybir.dt.float32r
AOP = mybir.AluOpType

N_CORES = 8
# per-dim taps: parity -> [(delta, k), ...]
TAPS = {0: [(-1, 3), (0, 1)], 1: [(0, 2), (1, 0)]}
PPS = ((0, 0), (0, 1), (1, 0), (1, 1))
NK = 17  # plane-pair tiles per core: k=0..16 holds local planes (k, k+1)

_PROG = None


def _widx(cg, s, ph, pw, t):
    """Flat index of the [128, 128] weight matrix for (cgroup, 64-ch strip,
    h/w polyphase, (dh, dw) tap index t in 0..3)."""
    return ((cg * 2 + s) * 4 + (ph * 2 + pw)) * 4 + t


def _build_program():
    nc = bacc.Bacc(
        "TRN2", target_bir_lowering=False, debug=False, num_devices=N_CORES
    )
    # xp: plane pairs, partition-ready: [k, q=(cg,s), j, cc, h, w]
    xp_d = nc.declare_dram_parameter("xp", [NK, 4, 2, 64, 32, 32], F32, isOutput=False)
    wd_d = nc.declare_dram_parameter("wpair", [128, 64, 128], F32, isOutput=False)
    # y: [k, cg, s, r, c', ph, pw, a, b]; plane l = 2k-1+r (r0 of k=0 and
    # r1 of k=16 are dropped by the host)
    y_d = nc.declare_dram_parameter(
        "y", [NK, 2, 2, 2, 64, 2, 2, 32, 32], F32, isOutput=True
    )

    with ExitStack() as ctx:
        tc = ctx.enter_context(tile.TileContext(nc))
        wpool = ctx.enter_context(tc.tile_pool(name="wpool", bufs=1))
        xpool = ctx.enter_context(tc.tile_pool(name="xpool", bufs=12))
        opool = ctx.enter_context(tc.tile_pool(name="opool", bufs=6))
        ppool = ctx.enter_context(tc.tile_pool(name="ppool", bufs=7, space="PSUM"))

        wd = wpool.tile([128, 64, 128], F32R)
        nc.sync.dma_start(out=wd[:], in_=wd_d[:, :, :].bitcast(F32R))

        def load_pair(k, cg, s):
            t = xpool.tile([128, 34, 34], F32R, tag="xp")
            # zero whole tile (halo border); interior DMA-overwritten
            nc.gpsimd.memset(t[:].bitcast(F32), 0.0)
            nc.sync.dma_start(
                out=t[:, 1:33, 1:33],
                in_=xp_d[k, cg * 2 + s]
                .rearrange("j c h w -> (j c) h w")
                .bitcast(F32R),
            )
            return t

        for k in range(NK):
            for cg in range(2):
                for s in range(2):
                    xt = load_pair(k, cg, s)
                    # out slab for 2 planes x 64 ch: [(r,c'), ph, pw, a, b]
                    ot = opool.tile([128, 2, 2, 32, 32], F32, tag="out")
                    for ph, pw in PPS:
                        hw_taps = [
                            (dh, kh, dw, kw)
                            for (dh, kh) in TAPS[ph]
                            for (dw, kw) in TAPS[pw]
                        ]
                        for ah in range(2):
                            a0 = ah * 16
                            ps = ppool.tile([128, 16, 32], F32, tag="ps")
                            for t_i, (dh, kh, dw, kw) in enumerate(hw_taps):
                                win = xt[
                                    :,
                                    1 + a0 + dh : 1 + a0 + dh + 16,
                                    1 + dw : 1 + dw + 32,
                                ]
                                wap = wd[:, _widx(cg, s, ph, pw, t_i), :]
                                nc.tensor.matmul(
                                    ps[:],
                                    wap,
                                    win,
                                    start=(t_i == 0),
                                    stop=(t_i == len(hw_taps) - 1),
                                )
                            nc.scalar.copy(ot[:, ph, pw, a0 : a0 + 16, :], ps[:])
                    nc.sync.dma_start(out=y_d[k, cg, s], in_=ot[:])
    nc.compile()
    return nc


def _get_program():
    global _PROG
    if _PROG is None:
        _PROG = _build_program()
    return _PROG


def _make_in_maps(x, filters):
    x = np.ascontiguousarray(np.asarray(x), dtype=np.float32)
    filters = np.asarray(filters, dtype=np.float32)
    ftap = filters[:, :, :, 0, :]  # (kd, kh, kw, c)

    # wpair[(j,cc), widx, (r,c')] = F[kd(j, parity(r)), kh, kw, cbase+c']
    #   * delta(cc, c');  r=0 -> parity 1 (l=2k-1), r=1 -> parity 0 (l=2k)
    wpair = np.zeros((128, 64, 128), np.float32)
    idx = np.arange(64)
    for cg in range(2):
        for s in range(2):
            cbase = cg * 128 + s * 64
            for ph, pw in PPS:
                taps = [(a, b) for a in TAPS[ph] for b in TAPS[pw]]
                for t, ((dh, kh), (dw, kw)) in enumerate(taps):
                    w = _widx(cg, s, ph, pw, t)
                    for r, pdr in ((0, 1), (1, 0)):
                        for j in range(2):
                            kd = TAPS[pdr][j][1]
                            wpair[j * 64 + idx, w, r * 64 + idx] = ftap[
                                kd, kh, kw, cbase : cbase + 64
                            ]

    in_maps = []
    for core in range(N_CORES):
        n, h = core // 2, core % 2
        lo = 16 * h - 1
        planes = np.zeros((18, 32, 32, 256), np.float32)
        s0, s1 = max(lo, 0), min(16 * h + 17, 32)
        planes[s0 - lo : s1 - lo] = x[n, s0:s1]
        planes = planes.transpose(0, 3, 1, 2)  # (18, 256, 32, 32)
        # pair planes: xp[k, q, j, cc, h, w] = planes[k+j, q*64+cc, h, w]
        pair = np.stack([planes[0:NK], planes[1 : NK + 1]], axis=1)
        pair = pair.reshape(NK, 2, 4, 64, 32, 32).transpose(0, 2, 1, 3, 4, 5)
        in_maps.append({"xp": np.ascontiguousarray(pair), "wpair": wpair})
    return in_maps


def kernel(x, filters):
    nc = _get_program()
    in_maps = _make_in_maps(x, filters)
    res = run_bass_kernel_spmd(nc, in_maps, list(range(N_CORES)))
    y = np.empty((4, 64, 64, 64, 256), np.float32)
    for core in range(N_CORES):
        n, h = core // 2, core % 2
        yc = res.results[core]["y"]  # (k, cg, s, r, c', p, q, a, b)
        # l = 2k-1+r; ho = 2a+p; wo = 2b+q; c = cg*128 + s*64 + c'
        yc = yc.transpose(0, 3, 7, 5, 8, 6, 1, 2, 4)  # (k,r,a,p,b,q,cg,s,c')
        yc = yc.reshape(2 * NK, 64, 64, 256)[1 : 2 * NK - 1]
        y[n, 32 * h : 32 * h + 32] = yc
    return y


# revision 11
# speedup vs baseline: 1.2005x; 1.1455x over previous
"""Depthwise 3D transposed conv (stride 2, k=4, SAME) on 8 trn2 NeuronCores.

x: (4, 32, 32, 32, 256) f32, filters: (4, 4, 4, 1, 256) f32
y: (4, 64, 64, 64, 256) f32

Sharding: 8 cores = (batch n in 4) x (d-halves in 2). Zero communication.

Math: polyphase decomposition of the stride-2 transposed conv. Per dim,
output parity p uses taps (delta, k):
  p=0: y[2m]   = f[3] x[m-1] + f[1] x[m]
  p=1: y[2m+1] = f[2] x[m]   + f[0] x[m+1]
Each output element is a sum of exactly 8 taps (2 per dim).

Compute (all TensorE, float32r = 1 cycle/row):
- Contraction folds the 2 d-taps: input tile partitions hold a PLANE PAIR,
  p = j*64 + cc <- (plane k+j, channel block cc of 64).
- Weight columns fold 2 OUTPUT PLANES: both output planes l=2k-1 (d-parity
  1) and l=2k (d-parity 0) read the same plane pair (k, k+1) with the same
  (dh, dw) window shifts, so a [128, 128] weight matrix with columns
  (r*64 + c'), W[(j,cc),(r,c')] = delta(cc,c') * F[kd(j, parity(r)), kh,
  kw, c'], computes partial sums for two planes in one matmul.
Each matmul thus covers 4 of the 8 taps for 2x64 channels x 512 positions
(256 useful MACs/cycle); 4 (dh,dw) taps accumulate per PSUM bank. ScalarE
evacuates PSUM->SBUF. ~2176 matmuls/core total.

Host pre-pairs planes (xp[k] = planes (k, k+1)) so every tile load is one
full-width 128-partition DMA with 4KB-contiguous per-partition runs. The
output accumulates in (plane-pair, polyphase)-major slabs stored as
contiguous [128, 16KB] DMAs; the host un-interleaves (and drops the two
out-of-range boundary plane slots).
"""
import sys

sys.path.insert(0, "/opt/trn_rl_repo")

from contextlib import ExitStack

import numpy as np

import concourse.bass as bass  # noqa: F401  (registers engine classes)
import concourse.tile as tile
from concourse import bacc, mybir
from concourse.bass_utils import run_bass_kernel_spmd

F32 = mybir.dt.float32
F32R = mybir.dt.float32r
AOP = mybir.AluOpType

N_CORES = 8
# per-dim taps: parity -> [(delta, k), ...]
TAPS = {0: [(-1, 3), (0, 1)], 1: [(0, 2), (1, 0)]}
PPS = ((0, 0), (0, 1), (1, 0), (1, 1))
NK = 17  # plane-pair tiles per core: k=0..16 holds local planes (k, k+1)

_PROG = None


def _widx(cg, s, ph, pw, t):
    """Flat index of the [128, 128] weight matrix for (cgroup, 64-ch strip,
    h/w polyphase, (dh, dw) tap index t in 0..3)."""
    return ((cg * 2 + s) * 4 + (ph * 2 + pw)) * 4 + t


def _build_program():
    nc = bacc.Bacc(
        "TRN2", target_bir_lowering=False, debug=False, num_devices=N_CORES
    )
    # xp: plane pairs, partition-ready: [k, q=(cg,s), j, cc, h, w]
    xp_d = nc.declare_dram_parameter("xp", [NK, 4, 2, 64, 34, 34], F32, isOutput=False)
    wd_d = nc.declare_dram_parameter("wpair", [128, 64, 128], F32, isOutput=False)
    # y: [k, cg, s, r, c', ph, pw, a, b]; plane l = 2k-1+r (r0 of k=0 and
    # r1 of k=16 are dropped by the host)
    y_d = nc.declare_dram_parameter(
        "y", [NK, 2, 2, 2, 64, 2, 2, 32, 32], F32, isOutput=True
    )

    with ExitStack() as ctx:
        tc = ctx.enter_context(tile.TileContext(nc))
        wpool = ctx.enter_context(tc.tile_pool(name="wpool", bufs=1))
        xpool = ctx.enter_context(tc.tile_pool(name="xpool", bufs=12))
        opool = ctx.enter_context(tc.tile_pool(name="opool", bufs=6))
        ppool = ctx.enter_context(tc.tile_pool(name="ppool", bufs=7, space="PSUM"))

        wd = wpool.tile([128, 64, 128], F32R)
        nc.sync.dma_start(out=wd[:], in_=wd_d[:, :, :].bitcast(F32R))

        def load_pair(k, cg, s):
            # halo border is pre-padded in DRAM: whole-tile contiguous load
            t = xpool.tile([128, 34, 34], F32R, tag="xp")
            nc.sync.dma_start(
                out=t[:],
                in_=xp_d[k, cg * 2 + s]
                .rearrange("j c h w -> (j c) h w")
                .bitcast(F32R),
            )
            return t

        for k in range(NK):
            for cg in range(2):
                for s in range(2):
                    xt = load_pair(k, cg, s)
                    # out slab for 2 planes x 64 ch: [(r,c'), ph, pw, a, b]
                    ot = opool.tile([128, 2, 2, 32, 32], F32, tag="out")
                    for ph, pw in PPS:
                        hw_taps = [
                            (dh, kh, dw, kw)
                            for (dh, kh) in TAPS[ph]
                            for (dw, kw) in TAPS[pw]
                        ]
                        for ah in range(2):
                            a0 = ah * 16
                            ps = ppool.tile([128, 16, 32], F32, tag="ps")
                            for t_i, (dh, kh, dw, kw) in enumerate(hw_taps):
                                win = xt[
                                    :,
                                    1 + a0 + dh : 1 + a0 + dh + 16,
                                    1 + dw : 1 + dw + 32,
                                ]
                                wap = wd[:, _widx(cg, s, ph, pw, t_i), :]
                                nc.tensor.matmul(
                                    ps[:],
                                    wap,
                                    win,
                                    start=(t_i == 0),
                                    stop=(t_i == len(hw_taps) - 1),
                                )
                            nc.scalar.copy(ot[:, ph, pw, a0 : a0 + 16, :], ps[:])
                    nc.sync.dma_start(out=y_d[k, cg, s], in_=ot[:])
    nc.compile()
    return nc


def _get_program():
    global _PROG
    if _PROG is None:
        _PROG = _build_program()
    return _PROG


def _make_in_maps(x, filters):
    x = np.ascontiguousarray(np.asarray(x), dtype=np.float32)
    filters = np.asarray(filters, dtype=np.float32)
    ftap = filters[:, :, :, 0, :]  # (kd, kh, kw, c)

    # wpair[(j,cc), widx, (r,c')] = F[kd(j, parity(r)), kh, kw, cbase+c']
    #   * delta(cc, c');  r=0 -> parity 1 (l=2k-1), r=1 -> parity 0 (l=2k)
    wpair = np.zeros((128, 64, 128), np.float32)
    idx = np.arange(64)
    for cg in range(2):
        for s in range(2):
            cbase = cg * 128 + s * 64
            for ph, pw in PPS:
                taps = [(a, b) for a in TAPS[ph] for b in TAPS[pw]]
                for t, ((dh, kh), (dw, kw)) in enumerate(taps):
                    w = _widx(cg, s, ph, pw, t)
                    for r, pdr in ((0, 1), (1, 0)):
                        for j in range(2):
                            kd = TAPS[pdr][j][1]
                            wpair[j * 64 + idx, w, r * 64 + idx] = ftap[
                                kd, kh, kw, cbase : cbase + 64
                            ]

    in_maps = []
    for core in range(N_CORES):
        n, h = core // 2, core % 2
        lo = 16 * h - 1
        planes = np.zeros((18, 32, 32, 256), np.float32)
        s0, s1 = max(lo, 0), min(16 * h + 17, 32)
        planes[s0 - lo : s1 - lo] = x[n, s0:s1]
        planes = planes.transpose(0, 3, 1, 2)  # (18, 256, 32, 32)
        # pair planes with zero halo: xp[k, q, j, cc, 1+h, 1+w] =
        # planes[k+j, q*64+cc, h, w]
        pair = np.stack([planes[0:NK], planes[1 : NK + 1]], axis=1)
        pair = pair.reshape(NK, 2, 4, 64, 32, 32).transpose(0, 2, 1, 3, 4, 5)
        padded = np.zeros((NK, 4, 2, 64, 34, 34), np.float32)
        padded[:, :, :, :, 1:33, 1:33] = pair
        in_maps.append({"xp": padded, "wpair": wpair})
    return in_maps


def kernel(x, filters):
    nc = _get_program()
    in_maps = _make_in_maps(x, filters)
    res = run_bass_kernel_spmd(nc, in_maps, list(range(N_CORES)))
    y = np.empty((4, 64, 64, 64, 256), np.float32)
    for core in range(N_CORES):
        n, h = core // 2, core % 2
        yc = res.results[core]["y"]  # (k, cg, s, r, c', p, q, a, b)
        # l = 2k-1+r; ho = 2a+p; wo = 2b+q; c = cg*128 + s*64 + c'
        yc = yc.transpose(0, 3, 7, 5, 8, 6, 1, 2, 4)  # (k,r,a,p,b,q,cg,s,c')
        yc = yc.reshape(2 * NK, 64, 64, 256)[1 : 2 * NK - 1]
        y[n, 32 * h : 32 * h + 32] = yc
    return y


# revision 12
# speedup vs baseline: 1.4101x; 1.1746x over previous
"""Depthwise 3D transposed conv (stride 2, k=4, SAME) on 8 trn2 NeuronCores.

x: (4, 32, 32, 32, 256) f32, filters: (4, 4, 4, 1, 256) f32
y: (4, 64, 64, 64, 256) f32

Sharding: 8 cores = (batch n in 4) x (d-halves in 2). Zero communication.

Math: polyphase decomposition of the stride-2 transposed conv. Per dim,
output parity p uses taps (delta, k):
  p=0: y[2m]   = f[3] x[m-1] + f[1] x[m]
  p=1: y[2m+1] = f[2] x[m]   + f[0] x[m+1]
Each output element is a sum of exactly 8 taps (2 per dim).

Compute (all TensorE, float32r = 1 cycle/row):
- Contraction folds the 2 d-taps: input tile partitions hold a PLANE PAIR,
  p = j*64 + cc <- (plane k+j, channel block cc of 64).
- Weight columns fold 2 OUTPUT PLANES: both output planes l=2k-1 (d-parity
  1) and l=2k (d-parity 0) read the same plane pair (k, k+1) with the same
  (dh, dw) window shifts, so a [128, 128] weight matrix with columns
  (r*64 + c'), W[(j,cc),(r,c')] = delta(cc,c') * F[kd(j, parity(r)), kh,
  kw, c'], computes partial sums for two planes in one matmul.
Each matmul thus covers 4 of the 8 taps for 2x64 channels x 512 positions
(256 useful MACs/cycle); 4 (dh,dw) taps accumulate per PSUM bank. ScalarE
evacuates PSUM->SBUF. ~2176 matmuls/core total.

Host pre-pairs planes (xp[k] = planes (k, k+1)) so every tile load is one
full-width 128-partition DMA with 4KB-contiguous per-partition runs. The
output accumulates in (plane-pair, polyphase)-major slabs stored as
contiguous [128, 16KB] DMAs; the host un-interleaves (and drops the two
out-of-range boundary plane slots).
"""
import sys

sys.path.insert(0, "/opt/trn_rl_repo")

from contextlib import ExitStack

import numpy as np

import concourse.bass as bass  # noqa: F401  (registers engine classes)
import concourse.tile as tile
from concourse import bacc, mybir
from concourse.bass_utils import run_bass_kernel_spmd

F32 = mybir.dt.float32
F32R = mybir.dt.float32r
AOP = mybir.AluOpType

N_CORES = 8
# per-dim taps: parity -> [(delta, k), ...]
TAPS = {0: [(-1, 3), (0, 1)], 1: [(0, 2), (1, 0)]}
PPS = ((0, 0), (0, 1), (1, 0), (1, 1))
NK = 17  # plane-pair tiles per core: k=0..16 holds local planes (k, k+1)

_PROG = None


def _widx(cg, s, ph, pw, t):
    """Flat index of the [128, 128] weight matrix for (cgroup, 64-ch strip,
    h/w polyphase, (dh, dw) tap index t in 0..3)."""
    return ((cg * 2 + s) * 4 + (ph * 2 + pw)) * 4 + t


def _build_program():
    nc = bacc.Bacc(
        "TRN2", target_bir_lowering=False, debug=False, num_devices=N_CORES
    )
    # xp: plane pairs, partition-ready: [k, q=(cg,s), j, cc, h, w]
    xp_d = nc.declare_dram_parameter("xp", [NK, 4, 2, 64, 34, 34], F32, isOutput=False)
    wd_d = nc.declare_dram_parameter("wpair", [128, 64, 128], F32, isOutput=False)
    # y: [k, cg, s, r, c', ph, pw, a, b]; plane l = 2k-1+r (r0 of k=0 and
    # r1 of k=16 are dropped by the host)
    y_d = nc.declare_dram_parameter(
        "y", [NK, 2, 2, 2, 64, 2, 2, 32, 32], F32, isOutput=True
    )

    with ExitStack() as ctx:
        tc = ctx.enter_context(tile.TileContext(nc))
        wpool = ctx.enter_context(tc.tile_pool(name="wpool", bufs=1))
        xpool = ctx.enter_context(tc.tile_pool(name="xpool", bufs=14))
        opool = ctx.enter_context(tc.tile_pool(name="opool", bufs=6))
        ppool = ctx.enter_context(tc.tile_pool(name="ppool", bufs=7, space="PSUM"))

        wd = wpool.tile([128, 64, 128], F32R)
        nc.sync.dma_start(out=wd[:], in_=wd_d[:, :, :].bitcast(F32R))

        def load_pair(k, cg, s):
            # halo border is pre-padded in DRAM: whole-tile contiguous load
            t = xpool.tile([128, 34, 34], F32R, tag="xp")
            nc.sync.dma_start(
                out=t[:],
                in_=xp_d[k, cg * 2 + s]
                .rearrange("j c h w -> (j c) h w")
                .bitcast(F32R),
            )
            return t

        for k in range(NK):
            for cg in range(2):
                for s in range(2):
                    xt = load_pair(k, cg, s)
                    # out slab for 2 planes x 64 ch: [(r,c'), ph, pw, a, b]
                    ot = opool.tile([128, 2, 2, 32, 32], F32, tag="out")
                    for ph, pw in PPS:
                        hw_taps = [
                            (dh, kh, dw, kw)
                            for (dh, kh) in TAPS[ph]
                            for (dw, kw) in TAPS[pw]
                        ]
                        for ah in range(2):
                            a0 = ah * 16
                            ps = ppool.tile([128, 16, 32], F32, tag="ps")
                            for t_i, (dh, kh, dw, kw) in enumerate(hw_taps):
                                win = xt[
                                    :,
                                    1 + a0 + dh : 1 + a0 + dh + 16,
                                    1 + dw : 1 + dw + 32,
                                ]
                                wap = wd[:, _widx(cg, s, ph, pw, t_i), :]
                                nc.tensor.matmul(
                                    ps[:],
                                    wap,
                                    win,
                                    start=(t_i == 0),
                                    stop=(t_i == len(hw_taps) - 1),
                                )
                            nc.scalar.copy(ot[:, ph, pw, a0 : a0 + 16, :], ps[:])
                    # SWDGE: keeps stores off the Sync FIFO so a
                    # blocked store never delays upcoming loads
                    nc.gpsimd.dma_start(out=y_d[k, cg, s], in_=ot[:])
    nc.compile()
    return nc


def _get_program():
    global _PROG
    if _PROG is None:
        _PROG = _build_program()
    return _PROG


def _make_in_maps(x, filters):
    x = np.ascontiguousarray(np.asarray(x), dtype=np.float32)
    filters = np.asarray(filters, dtype=np.float32)
    ftap = filters[:, :, :, 0, :]  # (kd, kh, kw, c)

    # wpair[(j,cc), widx, (r,c')] = F[kd(j, parity(r)), kh, kw, cbase+c']
    #   * delta(cc, c');  r=0 -> parity 1 (l=2k-1), r=1 -> parity 0 (l=2k)
    wpair = np.zeros((128, 64, 128), np.float32)
    idx = np.arange(64)
    for cg in range(2):
        for s in range(2):
            cbase = cg * 128 + s * 64
            for ph, pw in PPS:
                taps = [(a, b) for a in TAPS[ph] for b in TAPS[pw]]
                for t, ((dh, kh), (dw, kw)) in enumerate(taps):
                    w = _widx(cg, s, ph, pw, t)
                    for r, pdr in ((0, 1), (1, 0)):
                        for j in range(2):
                            kd = TAPS[pdr][j][1]
                            wpair[j * 64 + idx, w, r * 64 + idx] = ftap[
                                kd, kh, kw, cbase : cbase + 64
                            ]

    in_maps = []
    for core in range(N_CORES):
        n, h = core // 2, core % 2
        lo = 16 * h - 1
        planes = np.zeros((18, 32, 32, 256), np.float32)
        s0, s1 = max(lo, 0), min(16 * h + 17, 32)
        planes[s0 - lo : s1 - lo] = x[n, s0:s1]
        planes = planes.transpose(0, 3, 1, 2)  # (18, 256, 32, 32)
        # pair planes with zero halo: xp[k, q, j, cc, 1+h, 1+w] =
        # planes[k+j, q*64+cc, h, w]
        pair = np.stack([planes[0:NK], planes[1 : NK + 1]], axis=1)
        pair = pair.reshape(NK, 2, 4, 64, 32, 32).transpose(0, 2, 1, 3, 4, 5)
        padded = np.zeros((NK, 4, 2, 64, 34, 34), np.float32)
        padded[:, :, :, :, 1:33, 1:33] = pair
        in_maps.append({"xp": padded, "wpair": wpair})
    return in_maps


def kernel(x, filters):
    nc = _get_program()
    in_maps = _make_in_maps(x, filters)
    res = run_bass_kernel_spmd(nc, in_maps, list(range(N_CORES)))
    y = np.empty((4, 64, 64, 64, 256), np.float32)
    for core in range(N_CORES):
        n, h = core // 2, core % 2
        yc = res.results[core]["y"]  # (k, cg, s, r, c', p, q, a, b)
        # l = 2k-1+r; ho = 2a+p; wo = 2b+q; c = cg*128 + s*64 + c'
        yc = yc.transpose(0, 3, 7, 5, 8, 6, 1, 2, 4)  # (k,r,a,p,b,q,cg,s,c')
        yc = yc.reshape(2 * NK, 64, 64, 256)[1 : 2 * NK - 1]
        y[n, 32 * h : 32 * h + 32] = yc
    return y
